# revision 44
# baseline (speedup 1.0000x reference)
"""Trainium2 Bass kernel: causal GQA self-attention
(B=2, T=2048, C=1024, 16 q-heads / 4 kv-heads, rotary + q/k RMS-norm),
sharded over 8 NeuronCores as (batch x kv-group).

v2 redesign vs v1 (295-360us):
- fp8(e4m3) DoubleRow matmuls for QKV projection, P@V, and out-projection
  (2x PE throughput; contraction pairs ride the free dim so no partition
  reshuffle is needed).
- Phase 1 (proj+rope+rms) software-pipelined with attention: projections for
  query block j+1 are emitted interleaved with attention block j, so the
  DVE-bound rope/rms hides behind PE/ACT attention work.
- rope/rms in bf16 on DVE (2x perf mode), k-path on gpsimd.
- q/k transposes via DMA-transpose XBAR (off PE, no PSUM banks, no copies).
- exp merged per chunk-pair ([128,2,512] PSUM tiles spanning 2 banks):
  halves ACT instruction overhead; causal masking applied pre-exp on the
  f32 scores (triangle-only affine_select + gap memset on gpsimd).
- exp outputs fp8 with per-block bias (softmax shift-invariance makes the
  bias cancel in normalization), keeping P in e4m3 range.
- out-projection DMAs batched (one [1024,512] store per query block).
"""
import sys
from contextlib import ExitStack

for p in ("/opt/trn_rl_repo", "/root/.axon_site/_ro/trn_rl_repo"):
    if p not in sys.path:
        sys.path.insert(0, p)

import numpy as np
import ml_dtypes

import concourse.bass as bass
import concourse.mybir as mybir
from concourse.tile import TileContext

F32 = mybir.dt.float32
F32R = mybir.dt.float32r
BF16 = mybir.dt.bfloat16
FP8 = mybir.dt.float8e4
NPBF16 = ml_dtypes.bfloat16
NPFP8 = ml_dtypes.float8_e4m3
DR = mybir.MatmulPerfMode.DoubleRow

T, C, HQ, D = 2048, 1024, 4, 64
DQ = HQ * D          # 256 q dims per core
TC = T // 128        # 16 t-chunks
KC = C // 128        # 8 contraction chunks
NJ = T // 512        # 4 query blocks
EPS = 1.1920929e-7
NEG = -1e30
# per-query-block exp bias (softmax shift: exp(s/8 - C); cancels in the
# normalization). Keeps P in fp8e4 range for each block's typical rowmax.
CBIAS = (1.6, 3.0, 3.2, 3.3)


def _bcast_ap(sl, n, at=1):
    ap = list(sl.ap)
    ap.insert(at, [0, n])
    return bass.AP(tensor=sl.tensor, offset=sl.offset, ap=ap)


def _split_waits(nc, maxw=1):
    """Walrus in this toolchain allows 1 sem-wait per instruction; split extras
    onto preceding same-engine NoOps."""
    cnt = 0
    for f in nc.m.functions:
        for b in f.blocks:
            il = list(b.instructions)
            out = []
            changed = False
            for inst in il:
                si = inst.sync_info
                waits = list(si.on_wait) if si and si.on_wait else []
                if len(waits) > maxw:
                    chunks = [waits[i:i + maxw] for i in range(0, len(waits), maxw)]
                    for ch in chunks[:-1]:
                        cnt += 1
                        nop = mybir.InstNoOp(name=f"I-waitfix-{cnt}")
                        nop.engine = inst.engine
                        nop.sync_info = mybir.SyncInfo(on_wait=ch, on_update=[])
                        out.append(nop)
                    si.on_wait = chunks[-1]
                    inst.sync_info = si
                    changed = True
                out.append(inst)
            if changed:
                b.instructions = out
    return cnt


def _build_attn(ctx, tc, outs, ins):
    nc = tc.nc
    xT, wqkv, wo, cos2, sin2 = (
        ins["xT"], ins["wqkv"], ins["wo"], ins["cos2"], ins["sin2"])
    outT = outs["outT"]

    singles = ctx.enter_context(tc.tile_pool(name="singles", bufs=1))


    # ---- static inputs in SBUF ----
    xsb = singles.tile([128, KC, T], BF16, tag="xsb")
    xTr = xT.rearrange("(a p) t -> p a t", p=128)
    wqkv_sb = singles.tile([128, KC, DQ + 128], BF16, tag="wqkv_sb")
    nc.sync.dma_start(out=xsb[:, :, 0:512], in_=xTr[:, :, 0:512])
    nc.sync.dma_start(out=wqkv_sb, in_=wqkv.rearrange("(a p) n -> p a n", p=128))
    wo_sb = singles.tile([128, 2, C], BF16, tag="wo_sb")
    nc.sync.dma_start(out=wo_sb, in_=wo.rearrange("(a p) o -> p a o", p=128))
    cos_sb = singles.tile([128, TC, 64], BF16, tag="cos_sb")
    nc.sync.dma_start(out=cos_sb, in_=cos2.rearrange("(a p) d -> p a d", p=128))
    sin_sb = singles.tile([128, TC, 64], BF16, tag="sin_sb")
    nc.sync.dma_start(out=sin_sb, in_=sin2.rearrange("(a p) d -> p a d", p=128))
    for tq in range(1, 4):
        nc.sync.dma_start(out=xsb[:, :, tq * 512:(tq + 1) * 512],
                          in_=xTr[:, :, tq * 512:(tq + 1) * 512])

    # ---- persistent intermediates ----
    # qk2 packs q (4 heads) + k + k-dup per row so one rope/rms op chain
    # covers both: cols [0:256] q, [256:320] k, [320:384] k copy.
    qk2 = singles.tile([128, TC, 384], BF16, tag="qk2")
    v8 = singles.tile([128, TC, 72], BF16, tag="v8")      # v rows + ones col
    nc.vector.memset(v8[:, :, 64:65], 1.0)
    qtb = singles.tile([128, 2, T], BF16, tag="qtb")      # q^T by head pair
    kt2 = singles.tile([128, T], BF16, tag="kt2")         # k^T (x2 copies)
    y8 = singles.tile([128, 2, T], BF16, tag="y8")        # attn out (normed)

    qk5 = qk2.rearrange("p t (h d) -> p t h d", d=64)     # [.., 6, 64] view

    pools = (
        tc.tile_pool(name="app", bufs=2, space="PSUM"),
        tc.tile_pool(name="s1p", bufs=2, space="PSUM"),
        tc.tile_pool(name="o65p", bufs=2, space="PSUM"),
        tc.tile_pool(name="ptp", bufs=18),
        tc.tile_pool(name="ropep", bufs=2),
        tc.tile_pool(name="smallp", bufs=4),
        tc.tile_pool(name="osp", bufs=2),
    )
    app, s1p, o65p, ptp, ropep, smallp, osp = (
        ctx.enter_context(p) for p in pools)

    def proj_fillers(t_):
        """QKV projection for one t-chunk as a list of small closures, so the
        PE work can be drip-fed between attention score pairs."""
        qkv = app.tile([128, 512], F32, tag="a1")

        def mk(m0):
            def go():
                for m in (m0, m0 + 1):
                    nc.tensor.matmul(
                        qkv[:, 0:DQ + 128],
                        xsb[:, m, t_ * 128:(t_ + 1) * 128],
                        wqkv_sb[:, m, :],
                        start=(m == 0), stop=(m == KC - 1))
                if m0 == KC - 2:
                    nc.scalar.copy(qk2[:, t_, 0:DQ + 64], qkv[:, 0:DQ + 64])
                    nc.scalar.copy(v8[:, t_, 0:64],
                                   qkv[:, DQ + 64:DQ + 128])
            return go
        return [mk(m0) for m0 in range(0, KC, 2)]

    def emit_rope_rms(tb, half):
        """rope+rms for 2 t-chunks (q and k together), then DMA-transposes.
        rms via a DVE Newton rsqrt (mean(q^2) concentrates near 1, so a
        linear seed converges in 4 steps) -- no ACT Sqrt, so the ACT engine
        never switches activation tables away from exp/copy."""
        ts = 4 * tb + 2 * half
        te = ts + 2
        qk = qk5[:, ts:te, 0:5]                      # [128, 2, 5, 64]
        sq_ = ropep.tile([128, 2, 5, 64], BF16, tag="sq")
        nc.vector.tensor_mul(sq_, qk, qk)
        mv = smallp.tile([128, 2, 5], F32, tag="mv")
        nc.vector.tensor_reduce(mv, sq_, axis=mybir.AxisListType.X,
                                op=mybir.AluOpType.add)
        nc.vector.tensor_scalar_mul(mv, mv, 1.0 / D)
        rsq = smallp.tile([128, 2, 5], F32, tag="rsq")
        nt = smallp.tile([128, 2, 5], F32, tag="nt")
        nc.vector.tensor_scalar(rsq, mv, -0.5, 1.5,
                                mybir.AluOpType.mult, mybir.AluOpType.add)
        for _ in range(4):
            nc.vector.tensor_mul(nt, rsq, rsq)
            nc.vector.tensor_mul(nt, nt, mv)
            nc.vector.tensor_scalar(nt, nt, -0.5, 1.5,
                                    mybir.AluOpType.mult,
                                    mybir.AluOpType.add)
            nc.vector.tensor_mul(rsq, rsq, nt)
        # -- rope (DVE, bf16) --
        tmp = ropep.tile([128, 2, 5, 64], BF16, tag="tmp")
        nc.vector.tensor_mul(tmp[:, :, :, 0:32], qk[:, :, :, 32:64],
                             _bcast_ap(sin_sb[:, ts:te, 0:32], 5, at=2))
        nc.vector.tensor_mul(tmp[:, :, :, 32:64], qk[:, :, :, 0:32],
                             _bcast_ap(sin_sb[:, ts:te, 32:64], 5, at=2))
        nc.vector.tensor_mul(qk, qk, _bcast_ap(cos_sb[:, ts:te], 5, at=2))
        nc.vector.tensor_add(qk, qk, tmp)
        # -- apply rms, duplicate k for the quadrant layout --
        nc.vector.tensor_mul(qk, qk, _bcast_ap(rsq, D, at=3))
        nc.vector.tensor_copy(qk5[:, ts:te, 5], qk5[:, ts:te, 4])
        # -- transposes via DMA XBAR (3D out: both head pairs in one shot) --
        for t_ in range(ts, te):
            nc.sync.dma_start_transpose(
                qtb[:, :, t_ * 128:(t_ + 1) * 128], qk2[:, t_, 0:256])
            nc.sync.dma_start_transpose(
                kt2[:, t_ * 128:(t_ + 1) * 128], qk2[:, t_, 256:384])

    def emit_attn_head(j, h, fillers=()):
        """Attention for query block j, head h. Emits all scores+exps (with
        phase-1 filler closures drip-fed between pairs), then all P@V
        accumulations; returns a deferred normalization tail."""
        pair, base = h // 2, (h % 2) * 64
        tp = (base, 0) if base else None
        o65 = o65p.tile([128, 512], F32, tag="o65")
        npair = 2 * (j + 1)
        qsl0 = j * 512
        pts = []
        fillers = list(fillers)
        nfill = len(fillers)
        for m in range(npair):
            take = nfill * (m + 1) // npair - nfill * m // npair
            for _ in range(take):
                fillers.pop(0)()
            s1 = s1p.tile([128, 2, 512], F32, tag="s1")
            diag = m >= 2 * j
            qp = 0 if m <= 2 * j else 256
            for i in range(2):
                c = 2 * m + i
                # both chunks of a pair cover [qp:512] so the merged exp
                # never reads uninitialized PSUM; the second chunk's extra
                # 128 noncausal columns are never read by its AV matmul.
                nc.tensor.matmul(
                    s1[:, i, qp:512],
                    kt2[base:base + 64, c * 128:(c + 1) * 128],
                    qtb[base:base + 64, pair, qsl0 + qp:qsl0 + 512],
                    start=True, stop=True, tile_position=tp)
            pt = ptp.tile([128, 2, 512], BF16, tag="pt")
            nc.scalar.activation(pt[:, :, qp:512], s1[:, :, qp:512],
                                 mybir.ActivationFunctionType.Exp,
                                 scale=0.125)
            if diag:
                # zero the above-diagonal triangles post-exp; only the
                # (deferred) AV matmuls wait on these Pool ops.
                for i in range(2):
                    off = qp + i * 128
                    psl = pt[:, i, off:off + 128].bitcast(mybir.dt.int16)
                    nc.gpsimd.affine_select(
                        out=psl, in_=psl,
                        compare_op=mybir.AluOpType.is_ge,
                        fill=0, base=0,
                        pattern=[[1, 128]], channel_multiplier=-1)
            pts.append((pt, qp, m, diag))

        def emit_avs():
            # per-chunk matmuls; a diagonal pair's second chunk starts 128
            # columns later, so its dead region of pt is simply never read.
            for pt, qp, m, diag in pts:
                q1 = qp + 128 if diag else qp
                nc.tensor.matmul(
                    o65[0:65, qp:512], v8[:, 2 * m, 0:65],
                    pt[:, 0, qp:512], start=(m == 0), stop=False)
                nc.tensor.matmul(
                    o65[0:65, q1:512], v8[:, 2 * m + 1, 0:65],
                    pt[:, 1, q1:512],
                    start=False, stop=(m == npair - 1))

        def tail():
            # normalization: y = o65[0:64] / rowsum (row 64). Reciprocal on
            # DVE, row broadcast via a stride-0-partition SBUF->SBUF DMA
            # (off-engine), multiply on DVE (only one PSUM operand allowed).
            rec = smallp.tile([1, 512], F32, tag="rec")
            nc.vector.reciprocal(rec, o65[64:65, :])
            bcs = smallp.tile([64, 512], F32, tag="bcs")
            recb = bass.AP(tensor=rec.tensor, offset=rec.offset,
                           ap=[list(rec.ap[0]), [0, 64]] + list(rec.ap[1:]))
            nc.sync.dma_start(out=bcs, in_=recb)
            nc.vector.tensor_mul(y8[base:base + 64, pair, qsl0:qsl0 + 512],
                                 o65[0:64, :], bcs)
        return emit_avs, tail

    def emit_outproj(j):
        """out-projection for query block j (fp8 DoubleRow), one batched DMA."""
        osb = osp.tile([128, 8, 512], F32, tag="osb")
        outTr = outT.rearrange("(a p) t -> p a t", p=128)
        for m in range(8):
            ops_ = o65p.tile([128, 512], F32, tag="o65")
            for fc in range(2):
                nc.tensor.matmul(
                    ops_, wo_sb[:, fc, m * 128:(m + 1) * 128],
                    y8[:, fc, j * 512:(j + 1) * 512],
                    start=(fc == 0), stop=(fc == 1))
            nc.vector.tensor_copy(osb[:, m, :], ops_)
            if m in (3, 7):
                nc.sync.dma_start(
                    out=outTr[:, m - 3:m + 1, j * 512:(j + 1) * 512],
                    in_=osb[:, m - 3:m + 1, :])

    # ---- software-pipelined emission ----
    # Unit u = (j, h). Emission at step u: scores+exps(u), phase-1 piece for
    # block j+1 (projs at h0/h2, rope+rms+transposes at h1/h3), then the
    # u-2 normalization tail, then the u-1 AV matmuls. Deferring AVs one
    # unit keeps the ACT exp stream continuous across head boundaries;
    # deferring tails keeps PE off the reciprocal chain. Out-projection of
    # block j is emitted two units into block j+1.
    for t_ in range(4):
        for f in proj_fillers(t_):
            f()
    emit_rope_rms(0, 0)
    emit_rope_rms(0, 1)
    units = [(j, h) for j in range(NJ) for h in range(HQ)]
    avs_prev = tail_prev = tail_wait = None
    for j, h in units:
        fillers = ()
        if j < NJ - 1:
            tb = 4 * (j + 1)
            if h == 0:
                fillers = proj_fillers(tb + 0) + proj_fillers(tb + 1)
            elif h == 2:
                fillers = proj_fillers(tb + 2) + proj_fillers(tb + 3)
        avs, tail = emit_attn_head(j, h, fillers)
        if j < NJ - 1:
            if h == 1:
                emit_rope_rms(j + 1, 0)
            elif h == 3:
                emit_rope_rms(j + 1, 1)
        if tail_wait is not None:
            tail_wait()
        if avs_prev is not None:
            avs_prev()
        if h == 2 and j > 0:
            emit_outproj(j - 1)
        tail_wait = tail_prev
        avs_prev, tail_prev = avs, tail
    tail_wait()
    avs_prev()
    tail_prev()
    emit_outproj(NJ - 1)


def _build_nc(loop_n=0, split=True):
    """loop_n=0: single-shot kernel (grading path). loop_n=N>0: body wrapped
    in a hardware For-loop executing N times — used by test.py to measure
    per-iteration device time with launch overhead amortized out."""
    nc = bass.Bass("TRN2", target_bir_lowering=False, debug=False, num_devices=8)
    ins = {
        "xT": nc.dram_tensor("xT", [1024, 2048], BF16, kind="ExternalInput").ap(),
        "wqkv": nc.dram_tensor("wqkv", [1024, 384], BF16, kind="ExternalInput").ap(),
        "wo": nc.dram_tensor("wo", [256, 1024], BF16, kind="ExternalInput").ap(),
        "cos2": nc.dram_tensor("cos2", [2048, 64], BF16, kind="ExternalInput").ap(),
        "sin2": nc.dram_tensor("sin2", [2048, 64], BF16, kind="ExternalInput").ap(),
    }
    outs = {"outT": nc.dram_tensor("outT", [1024, 2048], F32,
                                   kind="ExternalOutput").ap()}
    with TileContext(nc) as tc:
        if loop_n:
            with tc.For_i(0, loop_n, 1,
                          hint_engines=(mybir.EngineType.PE,
                                        mybir.EngineType.Activation)):
                with ExitStack() as ctx:
                    _build_attn(ctx, tc, outs, ins)
        else:
            with ExitStack() as ctx:
                _build_attn(ctx, tc, outs, ins)
    if split:
        _split_waits(nc, maxw=1)
    return nc


def _shard_inputs(inputs, b, g):
    x, cos, sin = inputs["x"], inputs["cos"], inputs["sin"]
    Wq, Wk, Wv, Wo = inputs["Wq"], inputs["Wk"], inputs["Wv"], inputs["Wo"]
    qs, ks = slice(g * 256, (g + 1) * 256), slice(g * 64, (g + 1) * 64)
    c1 = np.asarray(cos[0, :, 0, :], dtype=np.float32)
    s1 = np.asarray(sin[0, :, 0, :], dtype=np.float32)
    wqkv = np.concatenate(
        [np.asarray(Wq[qs]).T, np.asarray(Wk[ks]).T, np.asarray(Wv[ks]).T],
        axis=1)
    return {
        "xT": np.ascontiguousarray(np.asarray(x[b]).T.astype(NPBF16)),
        "wqkv": np.ascontiguousarray(wqkv.astype(NPBF16)),
        "wo": np.ascontiguousarray(np.asarray(Wo[:, qs]).T.astype(NPBF16)),
        "cos2": np.ascontiguousarray(
            np.concatenate([c1, c1], axis=1).astype(NPBF16)),
        "sin2": np.ascontiguousarray(
            np.concatenate([s1, -s1], axis=1).astype(NPBF16)),
    }


_STATE = {}


def _get_state(loop_n=0):
    if loop_n in _STATE:
        return _STATE[loop_n]
    import jax
    from jax.sharding import Mesh, PartitionSpec, NamedSharding
    from jax.experimental.shard_map import shard_map
    from concourse.bass2jax import (
        _bass_exec_p, install_neuronx_cc_hook, partition_id_tensor)

    install_neuronx_cc_hook()
    nc = _build_nc(loop_n)
    pname = nc.partition_id_tensor.name if nc.partition_id_tensor else None

    in_names, out_names, out_avals, zero_outs = [], [], [], []
    for alloc in nc.m.functions[0].allocations:
        if not isinstance(alloc, mybir.MemoryLocationSet):
            continue
        name = alloc.memorylocations[0].name
        if alloc.kind == "ExternalInput":
            if name != pname:
                in_names.append(name)
        elif alloc.kind == "ExternalOutput":
            out_names.append(name)
            shape = tuple(alloc.tensor_shape)
            dtype = mybir.dt.np(alloc.dtype)
            out_avals.append(jax.core.ShapedArray(shape, dtype))
            zero_outs.append(np.zeros(shape, dtype))
    n_params = len(in_names)
    all_names = in_names + out_names
    if pname is not None:
        all_names = all_names + [pname]

    def _body(*args):
        operands = list(args)
        if pname is not None:
            operands.append(partition_id_tensor())
        outs = _bass_exec_p.bind(
            *operands, out_avals=tuple(out_avals), in_names=tuple(all_names),
            out_names=tuple(out_names), lowering_input_output_aliases=(),
            sim_require_finite=True, sim_require_nnan=True, nc=nc)
        return tuple(outs)

    devices = jax.devices()[:8]
    mesh = Mesh(np.asarray(devices), ("core",))
    specs = (PartitionSpec("core"),) * (n_params + 1)
    sharded = jax.jit(shard_map(_body, mesh=mesh, in_specs=specs,
                                out_specs=(PartitionSpec("core"),),
                                check_rep=False))
    sharding = NamedSharding(mesh, PartitionSpec("core"))
    zeros = jax.device_put(
        np.zeros((8 * 1024, 2048), np.float32), sharding)
    _STATE[loop_n] = dict(sharded=sharded, sharding=sharding,
                          in_names=in_names, zeros=zeros, jax=jax)
    return _STATE[loop_n]


def _run_device(in_maps, loop_n=0):
    st = _get_state(loop_n)
    jax = st["jax"]
    concat_in = [np.concatenate([m[n] for m in in_maps], axis=0)
                 for n in st["in_names"]]
    dev_in = [jax.device_put(a, st["sharding"]) for a in concat_in]
    out = st["sharded"](*dev_in, st["zeros"])[0]
    return np.asarray(out).reshape(8, 1024, 2048)


def kernel(**inputs) -> np.ndarray:
    inputs = {k: np.asarray(v) for k, v in inputs.items()}
    in_maps = [_shard_inputs(inputs, b, g) for b in range(2) for g in range(4)]
    arr = _run_device(in_maps)
    out = np.zeros((2, 2048, 1024), np.float32)
    for c in range(8):
        out[c // 4] += arr[c].T
    return out
